# revision 15
# baseline (speedup 1.0000x reference)
"""Trainium2 Bass kernel for the fixed CGP DAG elementwise model (v2).

Reference computation (per row of X, shape (B, 4), ephs shape (2,)):
    n4 = x0 * x1
    n5 = sin(n4 + c0)
    n6 = x2 * x3
    n7 = n5 * n6 + sin(x2)
    n8 = cos(n7) * c1 + x0
    out = stack([n7, n8], axis=1)          # (B, 2)

v2 design vs the v1 baseline (69-81us):
- Custom fused DVE ops (registered at import via the documented
  dve_ops.OPS extension point, numerics HW-verified):
    MUL_WRAP : w = wrap(x0*x1 + c0)   -- fuses the n4 mul into the wrap
    ADD2_WRAP: w = wrap(t7 + s2 + pi/2) -- computes the cos argument
               directly from (t7, s2) so o7 = t7+s2 becomes pure tail
               work on Pool and never blocks the DVE queue.
  Both are 1x ops (2194ns @2048) but each replaces a 2-op chain.
- Engine balance per tile (N=2048, all bf16 SBUF):
    DVE : MUL_WRAP(2194) wrap_x2(2194) n6(1127) t7(1127) ADD2_WRAP(2194)
          o8=stt(cs,c1,x0)(1127) = 9.96us  <- bottleneck
    ACT : sin x3 (2000 each) + fused output store trigger = 6.3us
    Pool: SWDGE fp8 cast-load + o7 = t7+s2 (4437) = ~5.5us
    SP  : bf16 pair load
- Inputs: x0,x2 stay bf16; x1,x3 stored as fp8 e3m4 (HW cast-during-DMA
  to bf16 is bit-exact incl. subnormals; e3m4 quantization of x1,x3
  raises rel err to ~8e-3 vs the 2e-2 gate). HBM traffic/core:
  6 MiB loads + 4 MiB stores = 10.5 MB (vs 12.6 bf16-only).
- DRAM layout is tile-packed [NT, planes, P, N] so every load/store is
  one contiguous 0.5-1MB DMA.
"""

import math
import sys

import numpy as np

if "/opt/trn_rl_repo" not in sys.path:
    sys.path.insert(0, "/opt/trn_rl_repo")

P = 128
B = 8388608
D = 4
N_CORES = 8
ROWS = B // N_CORES            # rows per core
TILE_N = 2048                  # rows per partition per tile
NT = ROWS // (P * TILE_N)      # tiles per core
PI = math.pi

# planes kept in bf16 vs quantized to fp8 e3m4 (indices into x0..x3)
BF_PLANES = (0, 2)
FP8_PLANES = (1, 3)
XBUFS, QBUFS, OBUFS, TBUFS = 4, 4, 4, 3
N6_POOL_FRAC = 0.0             # fraction of n6 computed on Pool (free-dim split)
O7_DVE_FRAC = 0.0              # fraction of o7 computed on DVE


def set_config(tile_n=None, bf_planes=None, fp8_planes=None, xbufs=None,
               qbufs=None, obufs=None, tbufs=None, n6_pool=None, o7_dve=None):
    global TILE_N, NT, BF_PLANES, FP8_PLANES, XBUFS, QBUFS, OBUFS, TBUFS
    global N6_POOL_FRAC, O7_DVE_FRAC, _CACHE
    if tile_n is not None:
        TILE_N = tile_n
        NT = ROWS // (P * TILE_N)
    if bf_planes is not None:
        BF_PLANES = tuple(bf_planes)
    if fp8_planes is not None:
        FP8_PLANES = tuple(fp8_planes)
    if xbufs is not None:
        XBUFS = xbufs
    if qbufs is not None:
        QBUFS = qbufs
    if obufs is not None:
        OBUFS = obufs
    if tbufs is not None:
        TBUFS = tbufs
    if n6_pool is not None:
        N6_POOL_FRAC = n6_pool
    if o7_dve is not None:
        O7_DVE_FRAC = o7_dve
    _CACHE = {}

_CACHE: dict = {}
_OPS_REGISTERED: dict = {}


def _register_custom_ops():
    """Register the fused wrap DveOps (documented extension point:
    dve_ops.OPS.append + sub-opcode row). Idempotent per process."""
    if _OPS_REGISTERED:
        return _OPS_REGISTERED
    from concourse.dve_ops import (
        DveOp, OPS, CUSTOM_DVE_SPECS, _SUB_OPCODE_FOR_NAME,
    )
    from concourse.dve_spec import Spec, Src0, Src1, C0, C1, C2, lower
    from concourse.dve_uop import DveOpSpec

    def mk(name, body, reference):
        if name in _SUB_OPCODE_FOR_NAME:
            return next(op for op in OPS if op.name == name)
        spec = Spec(body=body, reference=reference)
        row = max(_SUB_OPCODE_FOR_NAME.values()) + 1
        assert row < 0x20
        shas = {}
        for ver in ("v3", "v4"):
            s = DveOpSpec(name=name, opcode=row, uops=lower(spec, ver=ver),
                          rd1_en=True)
            shas[ver] = s.sha(ver)
        op = DveOp(name, spec, subdim=False, uops_sha=shas)
        OPS.append(op)
        _SUB_OPCODE_FOR_NAME[name] = row
        CUSTOM_DVE_SPECS[name] = spec
        return op

    _y = Src0 * Src1 + C0
    mw = mk(
        "ANT_MUL_RANGE_WRAP",
        _y + C2 * ((_y < -C1) - (_y > C1)),
        lambda in0, in1, s0, s1, imm2: (in0 * in1 + s0)
        + imm2 * (((in0 * in1 + s0) < -s1).astype(np.float32)
                  - ((in0 * in1 + s0) > s1).astype(np.float32)),
    )
    _z = Src0 + Src1 + C0
    aw = mk(
        "ANT_ADD2_RANGE_WRAP",
        _z + C2 * ((_z < -C1) - (_z > C1)),
        lambda in0, in1, s0, s1, imm2: (in0 + in1 + s0)
        + imm2 * (((in0 + in1 + s0) < -s1).astype(np.float32)
                  - ((in0 + in1 + s0) > s1).astype(np.float32)),
    )
    _OPS_REGISTERED["mw"] = mw
    _OPS_REGISTERED["aw"] = aw
    return _OPS_REGISTERED


def _emit_tiles(nc, tc, ctx, XBr, XQr, Or, c0: float, c1: float):
    import concourse.tile as tile  # noqa: F401
    from concourse import mybir

    ops = _register_custom_ops()
    mw, aw = ops["mw"], ops["aw"]
    bf16 = mybir.dt.bfloat16
    Act = mybir.ActivationFunctionType
    Alu = mybir.AluOpType

    N = TILE_N
    nb, nq = len(BF_PLANES), len(FP8_PLANES)
    xpool = ctx.enter_context(tc.tile_pool(name="xb", bufs=XBUFS))
    qpool = ctx.enter_context(tc.tile_pool(name="xq", bufs=QBUFS)) if nq else None
    opool = ctx.enter_context(tc.tile_pool(name="oo", bufs=OBUFS))
    tpool = ctx.enter_context(tc.tile_pool(name="tt", bufs=TBUFS))

    # free-dim split points (multiples of 16 keep DMA/2x alignment)
    o7_dve_n = (int(N * O7_DVE_FRAC) // 16) * 16
    n6_pool_n = (int(N * N6_POOL_FRAC) // 16) * 16

    # Software-pipelined emission: the cross-engine tail of tile t-1
    # (cs/u on ACT, o7/o8 on Pool, stores) is emitted during tile t, so no
    # engine queue ever waits on work issued later in its own iteration.
    prev = None  # state of tile t-1: dict(x0, t7, s2, w3)
    for t in range(NT + 1):
        cur = None
        if t < NT:
            xb = xpool.tile([P, nb * N], bf16, tag="xb")
            nc.sync.dma_start(out=xb[:], in_=XBr[t])
            if nq:
                xq = qpool.tile([P, nq * N], bf16, tag="xq")
                nc.gpsimd.dma_start(out=xq[:], in_=XQr[t])  # SWDGE cast

            def plane(i):
                if i in BF_PLANES:
                    j = BF_PLANES.index(i)
                    return xb[:, j * N:(j + 1) * N]
                j = FP8_PLANES.index(i)
                return xq[:, j * N:(j + 1) * N]

            x0, x1, x2, x3 = (plane(i) for i in range(4))

            # DVE prefix ops (no cross-engine inputs)
            w1 = tpool.tile([P, N], bf16, tag="w1")
            nc.vector._custom_dve(mw, out=w1[:], in0=x0, in1=x1, s0=c0,
                                  s1=PI, imm2=2 * PI)
            w2 = tpool.tile([P, N], bf16, tag="w2")
            nc.vector.add_range_wrap(w2[:], x2, shift=0.0, bound=PI,
                                     period=2 * PI)
            n6 = tpool.tile([P, N], bf16, tag="n6")
            if n6_pool_n:
                nc.gpsimd.tensor_mul(n6[:, :n6_pool_n], x2[:, :n6_pool_n],
                                     x3[:, :n6_pool_n])
                nc.vector.tensor_mul(n6[:, n6_pool_n:], x2[:, n6_pool_n:],
                                     x3[:, n6_pool_n:])
            else:
                nc.vector.tensor_mul(n6[:], x2, x3)

        if prev is not None:
            # ACT tail of t-1 first in this iteration's ACT queue
            cs = tpool.tile([P, N], bf16, tag="cs")
            nc.scalar.activation(cs[:], prev["w3"][:], Act.Sin)

        if t < NT:
            n5 = tpool.tile([P, N], bf16, tag="n5")
            nc.scalar.activation(n5[:], w1[:], Act.Sin)
            s2 = tpool.tile([P, N], bf16, tag="s2")
            nc.scalar.activation(s2[:], w2[:], Act.Sin)

        if prev is not None:
            u = tpool.tile([P, N], bf16, tag="u")
            nc.scalar.mul(u[:], cs[:], c1)
            # Pool tail of t-1
            o7 = opool.tile([P, N], bf16, tag="o7")
            pt7, ps2 = prev["t7"], prev["s2"]
            if o7_dve_n:
                nc.vector.tensor_add(o7[:, :o7_dve_n], pt7[:, :o7_dve_n],
                                     ps2[:, :o7_dve_n])
            if o7_dve_n < N:
                nc.gpsimd.tensor_add(o7[:, o7_dve_n:], pt7[:, o7_dve_n:],
                                     ps2[:, o7_dve_n:])
            o8 = opool.tile([P, N], bf16, tag="o8")
            nc.gpsimd.tensor_add(o8[:], u[:], prev["x0"])
            nc.scalar.dma_start(out=Or[t - 1, 0], in_=o7[:])
            nc.gpsimd.dma_start(out=Or[t - 1, 1], in_=o8[:])

        if t < NT:
            # DVE dependent ops (inputs produced by ACT earlier this iter)
            t7 = tpool.tile([P, N], bf16, tag="t7")
            nc.vector.tensor_mul(t7[:], n5[:], n6[:])
            w3 = tpool.tile([P, N], bf16, tag="w3")
            nc.vector._custom_dve(aw, out=w3[:], in0=t7[:], in1=s2[:],
                                  s0=PI / 2, s1=PI, imm2=2 * PI)
            cur = {"x0": x0, "t7": t7, "s2": s2, "w3": w3}
        prev = cur


def _build_bass(c0: float, c1: float, reps: int | None = None):
    from contextlib import ExitStack

    import concourse.tile as tile
    from concourse import bacc, mybir

    _register_custom_ops()
    bf16 = mybir.dt.bfloat16
    f8 = mybir.dt.float8e3
    nb, nq = len(BF_PLANES), len(FP8_PLANES)

    nc = bacc.Bacc()
    XB = nc.declare_dram_parameter("XB", [NT, P, nb * TILE_N], bf16,
                                   isOutput=False)
    XQ = (nc.declare_dram_parameter("XQ", [NT, P, nq * TILE_N], f8,
                                    isOutput=False) if nq else None)
    O = nc.declare_dram_parameter("out", [NT, 2, P, TILE_N], bf16,
                                  isOutput=True)
    XBr = XB[:]
    XQr = XQ[:] if nq else None
    Or = O[:]

    with tile.TileContext(nc) as tc, ExitStack() as ctx:
        if reps is None:
            _emit_tiles(nc, tc, ctx, XBr, XQr, Or, c0, c1)
        else:
            with tc.For_i(0, reps, 1):
                _emit_tiles(nc, tc, ctx, XBr, XQr, Or, c0, c1)

    nc.compile()
    return nc


def _get_nc(c0: float, c1: float):
    key = (round(c0, 9), round(c1, 9), TILE_N, BF_PLANES, FP8_PLANES,
           XBUFS, QBUFS, OBUFS, TBUFS, N6_POOL_FRAC, O7_DVE_FRAC)
    if key not in _CACHE:
        _CACHE[key] = _build_bass(c0, c1)
    return _CACHE[key]


def prepare_in_map(X_core: np.ndarray) -> dict:
    """X_core: [ROWS, 4] f32 -> tile-packed DRAM tensors for one core.
    Layout: [NT, P, nplanes*TILE_N] with planes concatenated per partition
    row, so every tile load is one fully-contiguous DMA."""
    import ml_dtypes

    bf16 = ml_dtypes.bfloat16
    e3m4 = ml_dtypes.float8_e3m4
    m = {}
    planes = X_core.T.reshape(4, NT, P, TILE_N)
    if BF_PLANES:
        m["XB"] = np.ascontiguousarray(
            np.concatenate([planes[i] for i in BF_PLANES], axis=2)).astype(bf16)
    if FP8_PLANES:
        m["XQ"] = np.ascontiguousarray(
            np.concatenate([planes[i] for i in FP8_PLANES], axis=2)).astype(e3m4)
    return m


def kernel(X, ephs):
    from concourse.bass_utils import run_bass_kernel_spmd

    X = np.asarray(X, dtype=np.float32)
    ephs = np.asarray(ephs, dtype=np.float32).reshape(2)
    assert X.shape == (B, D), X.shape

    nc = _get_nc(float(ephs[0]), float(ephs[1]))
    in_maps = [prepare_in_map(X[i * ROWS:(i + 1) * ROWS])
               for i in range(N_CORES)]
    res = run_bass_kernel_spmd(nc, in_maps, list(range(N_CORES)))

    out = np.empty((B, 2), dtype=np.float32)
    for i in range(N_CORES):
        r = res.results[i]["out"]  # [NT, 2, P, TILE_N] bf16
        out[i * ROWS:(i + 1) * ROWS, 0] = (
            r[:, 0].astype(np.float32).reshape(ROWS))
        out[i * ROWS:(i + 1) * ROWS, 1] = (
            r[:, 1].astype(np.float32).reshape(ROWS))
    return out


# revision 18
# speedup vs baseline: 1.4429x; 1.4429x over previous
"""Trainium2 Bass kernel for the fixed CGP DAG elementwise model (v2).

Reference computation (per row of X, shape (B, 4), ephs shape (2,)):
    n4 = x0 * x1
    n5 = sin(n4 + c0)
    n6 = x2 * x3
    n7 = n5 * n6 + sin(x2)
    n8 = cos(n7) * c1 + x0
    out = stack([n7, n8], axis=1)          # (B, 2)

v2 design vs the v1 baseline (69-81us):
- Custom fused DVE ops (registered at import via the documented
  dve_ops.OPS extension point, numerics HW-verified):
    MUL_WRAP : w = wrap(x0*x1 + c0)   -- fuses the n4 mul into the wrap
    ADD2_WRAP: w = wrap(t7 + s2 + pi/2) -- computes the cos argument
               directly from (t7, s2) so o7 = t7+s2 becomes pure tail
               work on Pool and never blocks the DVE queue.
  Both are 1x ops (2194ns @2048) but each replaces a 2-op chain.
- Engine balance per tile (N=2048, all bf16 SBUF):
    DVE : MUL_WRAP(2194) wrap_x2(2194) n6(1127) t7(1127) ADD2_WRAP(2194)
          o8=stt(cs,c1,x0)(1127) = 9.96us  <- bottleneck
    ACT : sin x3 (2000 each) + fused output store trigger = 6.3us
    Pool: SWDGE fp8 cast-load + o7 = t7+s2 (4437) = ~5.5us
    SP  : bf16 pair load
- Inputs: x0,x2 stay bf16; x1,x3 stored as fp8 e3m4 (HW cast-during-DMA
  to bf16 is bit-exact incl. subnormals; e3m4 quantization of x1,x3
  raises rel err to ~8e-3 vs the 2e-2 gate). HBM traffic/core:
  6 MiB loads + 4 MiB stores = 10.5 MB (vs 12.6 bf16-only).
- DRAM layout is tile-packed [NT, planes, P, N] so every load/store is
  one contiguous 0.5-1MB DMA.
"""

import math
import sys

import numpy as np

if "/opt/trn_rl_repo" not in sys.path:
    sys.path.insert(0, "/opt/trn_rl_repo")

P = 128
B = 8388608
D = 4
N_CORES = 8
ROWS = B // N_CORES            # rows per core
TILE_N = 2048                  # rows per partition per tile
NT = ROWS // (P * TILE_N)      # tiles per core
PI = math.pi

# planes kept in bf16 vs quantized to fp8 e3m4 (indices into x0..x3)
BF_PLANES = (0, 2)
FP8_PLANES = (1, 3)
XBUFS, QBUFS, OBUFS, TBUFS = 4, 4, 4, 4
N6_POOL_FRAC = 0.0             # fraction of n6 computed on Pool (free-dim split)
O7_DVE_FRAC = 0.0              # fraction of o7 computed on DVE


def set_config(tile_n=None, bf_planes=None, fp8_planes=None, xbufs=None,
               qbufs=None, obufs=None, tbufs=None, n6_pool=None, o7_dve=None):
    global TILE_N, NT, BF_PLANES, FP8_PLANES, XBUFS, QBUFS, OBUFS, TBUFS
    global N6_POOL_FRAC, O7_DVE_FRAC, _CACHE
    if tile_n is not None:
        TILE_N = tile_n
        NT = ROWS // (P * TILE_N)
    if bf_planes is not None:
        BF_PLANES = tuple(bf_planes)
    if fp8_planes is not None:
        FP8_PLANES = tuple(fp8_planes)
    if xbufs is not None:
        XBUFS = xbufs
    if qbufs is not None:
        QBUFS = qbufs
    if obufs is not None:
        OBUFS = obufs
    if tbufs is not None:
        TBUFS = tbufs
    if n6_pool is not None:
        N6_POOL_FRAC = n6_pool
    if o7_dve is not None:
        O7_DVE_FRAC = o7_dve
    _CACHE = {}

_CACHE: dict = {}
_OPS_REGISTERED: dict = {}


def _register_custom_ops():
    """Register the fused wrap DveOps (documented extension point:
    dve_ops.OPS.append + sub-opcode row). Idempotent per process."""
    if _OPS_REGISTERED:
        return _OPS_REGISTERED
    from concourse.dve_ops import (
        DveOp, OPS, CUSTOM_DVE_SPECS, _SUB_OPCODE_FOR_NAME,
    )
    from concourse.dve_spec import Spec, Src0, Src1, C0, C1, C2, lower
    from concourse.dve_uop import DveOpSpec

    def mk(name, body, reference):
        if name in _SUB_OPCODE_FOR_NAME:
            return next(op for op in OPS if op.name == name)
        spec = Spec(body=body, reference=reference)
        row = max(_SUB_OPCODE_FOR_NAME.values()) + 1
        assert row < 0x20
        shas = {}
        for ver in ("v3", "v4"):
            s = DveOpSpec(name=name, opcode=row, uops=lower(spec, ver=ver),
                          rd1_en=True)
            shas[ver] = s.sha(ver)
        op = DveOp(name, spec, subdim=False, uops_sha=shas)
        OPS.append(op)
        _SUB_OPCODE_FOR_NAME[name] = row
        CUSTOM_DVE_SPECS[name] = spec
        return op

    _y = Src0 * Src1 + C0
    mw = mk(
        "ANT_MUL_RANGE_WRAP",
        _y + C2 * ((_y < -C1) - (_y > C1)),
        lambda in0, in1, s0, s1, imm2: (in0 * in1 + s0)
        + imm2 * (((in0 * in1 + s0) < -s1).astype(np.float32)
                  - ((in0 * in1 + s0) > s1).astype(np.float32)),
    )
    _z = Src0 + Src1 + C0
    aw = mk(
        "ANT_ADD2_RANGE_WRAP",
        _z + C2 * ((_z < -C1) - (_z > C1)),
        lambda in0, in1, s0, s1, imm2: (in0 + in1 + s0)
        + imm2 * (((in0 + in1 + s0) < -s1).astype(np.float32)
                  - ((in0 + in1 + s0) > s1).astype(np.float32)),
    )
    _OPS_REGISTERED["mw"] = mw
    _OPS_REGISTERED["aw"] = aw
    return _OPS_REGISTERED


def _emit_tiles(nc, tc, ctx, XBr, XQr, Or, c0: float, c1: float):
    import concourse.tile as tile  # noqa: F401
    from concourse import mybir

    ops = _register_custom_ops()
    mw, aw = ops["mw"], ops["aw"]
    bf16 = mybir.dt.bfloat16
    Act = mybir.ActivationFunctionType
    Alu = mybir.AluOpType

    N = TILE_N
    nb, nq = len(BF_PLANES), len(FP8_PLANES)
    xpool = ctx.enter_context(tc.tile_pool(name="xb", bufs=XBUFS))
    qpool = ctx.enter_context(tc.tile_pool(name="xq", bufs=QBUFS)) if nq else None
    opool = ctx.enter_context(tc.tile_pool(name="oo", bufs=OBUFS))
    # long-lived temps (live 2-3 pipeline stages) vs short-lived (same iter)
    tpool = ctx.enter_context(tc.tile_pool(name="tl", bufs=TBUFS))
    spool = ctx.enter_context(tc.tile_pool(name="ts", bufs=2))

    # free-dim split points (multiples of 16 keep DMA/2x alignment)
    o7_dve_n = (int(N * O7_DVE_FRAC) // 16) * 16
    n6_pool_n = (int(N * N6_POOL_FRAC) // 16) * 16

    # Two-stage software pipeline. Stages per emission iteration t:
    #   DVE : w1(t) w2(t) n6(t) | t7(t-1) w3(t-1)   <- consumes ACT of t-1
    #   ACT : cs(t-2) n5(t) s2(t) u(t-2)
    #   Pool: o7(t-2) o8(t-2) + stores(t-2)
    # so every engine-queue op only needs results emitted >= 1 iteration
    # earlier on other engines: no in-order queue ever stalls mid-iteration.
    s1_, s2_ = None, None  # pipeline state for tiles t-1 and t-2
    for t in range(NT + 2):
        cur = None
        if t < NT:
            xb = xpool.tile([P, nb * N], bf16, tag="xb")
            nc.sync.dma_start(out=xb[:], in_=XBr[t])
            if nq:
                xq = qpool.tile([P, nq * N], bf16, tag="xq")
                nc.gpsimd.dma_start(out=xq[:], in_=XQr[t])  # SWDGE cast

            def plane(i):
                if i in BF_PLANES:
                    j = BF_PLANES.index(i)
                    return xb[:, j * N:(j + 1) * N]
                j = FP8_PLANES.index(i)
                return xq[:, j * N:(j + 1) * N]

            x0, x1, x2, x3 = (plane(i) for i in range(4))

            # DVE prefix ops (no cross-engine inputs)
            w1 = spool.tile([P, N], bf16, tag="w1")
            nc.vector._custom_dve(mw, out=w1[:], in0=x0, in1=x1, s0=c0,
                                  s1=PI, imm2=2 * PI)
            w2 = spool.tile([P, N], bf16, tag="w2")
            nc.vector.add_range_wrap(w2[:], x2, shift=0.0, bound=PI,
                                     period=2 * PI)
            n6 = tpool.tile([P, N], bf16, tag="n6")
            if n6_pool_n:
                nc.gpsimd.tensor_mul(n6[:, :n6_pool_n], x2[:, :n6_pool_n],
                                     x3[:, :n6_pool_n])
                nc.vector.tensor_mul(n6[:, n6_pool_n:], x2[:, n6_pool_n:],
                                     x3[:, n6_pool_n:])
            else:
                nc.vector.tensor_mul(n6[:], x2, x3)
            cur = {"x0": x0, "n5": None, "n6": n6}

        if s2_ is not None:
            # ACT tail of t-2 heads this iteration's ACT queue (inputs old)
            cs = spool.tile([P, N], bf16, tag="cs")
            nc.scalar.activation(cs[:], s2_["w3"][:], Act.Sin)

        if t < NT:
            n5 = tpool.tile([P, N], bf16, tag="n5")
            nc.scalar.activation(n5[:], w1[:], Act.Sin)
            s2t = tpool.tile([P, N], bf16, tag="s2")
            nc.scalar.activation(s2t[:], w2[:], Act.Sin)
            cur["n5"], cur["s2"] = n5, s2t

        if s2_ is not None:
            u = spool.tile([P, N], bf16, tag="u")
            nc.scalar.mul(u[:], cs[:], c1)
            # Pool tail of t-2
            o7 = opool.tile([P, N], bf16, tag="o7")
            pt7, ps2 = s2_["t7"], s2_["s2"]
            if o7_dve_n:
                nc.vector.tensor_add(o7[:, :o7_dve_n], pt7[:, :o7_dve_n],
                                     ps2[:, :o7_dve_n])
            if o7_dve_n < N:
                nc.gpsimd.tensor_add(o7[:, o7_dve_n:], pt7[:, o7_dve_n:],
                                     ps2[:, o7_dve_n:])
            o8 = opool.tile([P, N], bf16, tag="o8")
            nc.gpsimd.tensor_add(o8[:], u[:], s2_["x0"])
            nc.scalar.dma_start(out=Or[t - 2, 0], in_=o7[:])
            nc.gpsimd.dma_start(out=Or[t - 2, 1], in_=o8[:])

        if s1_ is not None:
            # DVE stage-2 ops of t-1 (consume ACT outputs of t-1, emitted
            # one iteration ago -> no stall)
            t7 = tpool.tile([P, N], bf16, tag="t7")
            nc.vector.tensor_mul(t7[:], s1_["n5"][:], s1_["n6"][:])
            w3 = tpool.tile([P, N], bf16, tag="w3")
            nc.vector._custom_dve(aw, out=w3[:], in0=t7[:], in1=s1_["s2"][:],
                                  s0=PI / 2, s1=PI, imm2=2 * PI)
            s1_["t7"], s1_["w3"] = t7, w3

        s1_, s2_ = cur, s1_


def _build_bass(c0: float, c1: float, reps: int | None = None):
    from contextlib import ExitStack

    import concourse.tile as tile
    from concourse import bacc, mybir

    _register_custom_ops()
    bf16 = mybir.dt.bfloat16
    f8 = mybir.dt.float8e3
    nb, nq = len(BF_PLANES), len(FP8_PLANES)

    nc = bacc.Bacc()
    XB = nc.declare_dram_parameter("XB", [NT, P, nb * TILE_N], bf16,
                                   isOutput=False)
    XQ = (nc.declare_dram_parameter("XQ", [NT, P, nq * TILE_N], f8,
                                    isOutput=False) if nq else None)
    O = nc.declare_dram_parameter("out", [NT, 2, P, TILE_N], bf16,
                                  isOutput=True)
    XBr = XB[:]
    XQr = XQ[:] if nq else None
    Or = O[:]

    with tile.TileContext(nc) as tc, ExitStack() as ctx:
        if reps is None:
            _emit_tiles(nc, tc, ctx, XBr, XQr, Or, c0, c1)
        else:
            with tc.For_i(0, reps, 1):
                _emit_tiles(nc, tc, ctx, XBr, XQr, Or, c0, c1)

    nc.compile()
    return nc


def _get_nc(c0: float, c1: float):
    key = (round(c0, 9), round(c1, 9), TILE_N, BF_PLANES, FP8_PLANES,
           XBUFS, QBUFS, OBUFS, TBUFS, N6_POOL_FRAC, O7_DVE_FRAC)
    if key not in _CACHE:
        _CACHE[key] = _build_bass(c0, c1)
    return _CACHE[key]


def prepare_in_map(X_core: np.ndarray) -> dict:
    """X_core: [ROWS, 4] f32 -> tile-packed DRAM tensors for one core.
    Layout: [NT, P, nplanes*TILE_N] with planes concatenated per partition
    row, so every tile load is one fully-contiguous DMA."""
    import ml_dtypes

    bf16 = ml_dtypes.bfloat16
    e3m4 = ml_dtypes.float8_e3m4
    m = {}
    planes = X_core.T.reshape(4, NT, P, TILE_N)
    if BF_PLANES:
        m["XB"] = np.ascontiguousarray(
            np.concatenate([planes[i] for i in BF_PLANES], axis=2)).astype(bf16)
    if FP8_PLANES:
        m["XQ"] = np.ascontiguousarray(
            np.concatenate([planes[i] for i in FP8_PLANES], axis=2)).astype(e3m4)
    return m


def kernel(X, ephs):
    from concourse.bass_utils import run_bass_kernel_spmd

    X = np.asarray(X, dtype=np.float32)
    ephs = np.asarray(ephs, dtype=np.float32).reshape(2)
    assert X.shape == (B, D), X.shape

    nc = _get_nc(float(ephs[0]), float(ephs[1]))
    in_maps = [prepare_in_map(X[i * ROWS:(i + 1) * ROWS])
               for i in range(N_CORES)]
    res = run_bass_kernel_spmd(nc, in_maps, list(range(N_CORES)))

    out = np.empty((B, 2), dtype=np.float32)
    for i in range(N_CORES):
        r = res.results[i]["out"]  # [NT, 2, P, TILE_N] bf16
        out[i * ROWS:(i + 1) * ROWS, 0] = (
            r[:, 0].astype(np.float32).reshape(ROWS))
        out[i * ROWS:(i + 1) * ROWS, 1] = (
            r[:, 1].astype(np.float32).reshape(ROWS))
    return out
